# revision 1
# baseline (speedup 1.0000x reference)
"""Trainium2 Bass kernel for CenterDependentPool2D.

Input  x: (8, 64, 448, 448) fp32  ->  Output: (8, 64, 224, 224) fp32.

Strategy (per core = one batch element, 64 channels):
  - Partition p = c + 64*wg: channel c, wg = column half (0: out cols 0..111,
    1: out cols 112..223). Both pooling axes live in the free dimension, so
    every shift is a legal free-dim AP offset.
  - All five ring windows (k in {2,8,14,20,26}, stride 2, reflect pad)
    decompose over pair-max arrays:  E[j]=max(x[2j],x[2j+1]) (even pairs)
    and O[j]=max(x[2j+1],x[2j+2]) (odd pairs), in both H and W.  Ring r's
    window is then an s x s stride-1 square over EE (rings 1/3/5, s=1/7/13)
    or OO (rings 2/4, s=4/10), computed with a shared shifted-max pyramid
    (S2->S4->S8 + one combine per ring).
  - Reflect padding == window clipping here (pad 12/13 < window extent), so
    out-of-range leaves are just -BIG fills; no mirrored copies needed.
  - Ring blend: nested-disk masks (d2 < R^2), applied as a chain of
    copy_predicated overwrites P5 <- P4 <- P3 <- P2 <- P1 (windows nest, so
    later overwrites only shrink the window).
  - Pipeline in bf/fp16 after the first max (monotone rounding: fp16 pool =
    fp16(fp32 pool)); final cast back to fp32 on the scalar engine.
"""

import numpy as np

import concourse.bass as bass
import concourse.mybir as mybir
from concourse.tile import TileContext
from concourse.bass_utils import run_bass_kernel_spmd

# ---------------- problem constants ----------------
B, C, IN, OUT = 8, 64, 448, 224
OW = 112          # out cols per wg
EW = 124          # e-column count of pair arrays
WIN = 250         # input tile cols (incl pads)
NEG = -30000.0    # "minus infinity" that survives fp16
R2 = [60 * 60, 75 * 75, 90 * 90, 105 * 105]

DT = mybir.dt.float16      # pipeline dtype after first max
USE_GPSIMD = False         # offload Ow/OO to GPSIMD

_CACHED = {}


def _build_masks() -> np.ndarray:
    """RMASK [128, 4, 224, 112] u8: nested-disk masks per partition.

    ridx 0..3 = inside disk R4(105), R3(90), R2(75), R1(60) -> overwrite
    order P4, P3, P2, P1."""
    yy, xx = np.mgrid[0:OUT, 0:OUT]
    d2 = (yy - OUT // 2) ** 2 + (xx - OUT // 2) ** 2
    nested = np.stack([(d2 < R2[3]), (d2 < R2[2]), (d2 < R2[1]), (d2 < R2[0])])
    nested = nested.astype(np.uint8)  # [4, 224, 224]
    rm = np.zeros((128, 4, OUT, OW), np.uint8)
    for p in range(128):
        wg = p // 64
        rm[p] = nested[:, :, wg * OW:(wg + 1) * OW]
    return rm


def split_multi_waits(nc):
    """walrus CoreV3Gen accepts at most 1 sync-wait per instruction; Tile's
    tail drains can carry 2+.  Peel extras onto preceding NoOps."""
    n = 0
    for fn in nc.m.functions:
        for bb in fn.blocks:
            insts = list(bb.instructions)
            out = []
            for ins in insts:
                si = getattr(ins, "sync_info", None)
                if si is not None and len(si.on_wait) > 1:
                    waits = list(si.on_wait)
                    for k, w in enumerate(waits[:-1]):
                        nop = mybir.InstNoOp(
                            name=f"{ins.name}-waitsplit{k}",
                            engine=ins.engine, ins=[], outs=[])
                        nop.sync_info = mybir.SyncInfo(on_wait=[w], on_update=[])
                        out.append(nop)
                        n += 1
                    ins.sync_info = mybir.SyncInfo(
                        on_wait=[waits[-1]], on_update=list(si.on_update))
                out.append(ins)
            if n:
                bb.instructions = out
    return n


def _emit_kernel(nc: bass.Bass):
    x = nc.dram_tensor("x", [C, IN, IN], mybir.dt.float32, kind="ExternalInput")
    y = nc.dram_tensor("y", [C, OUT, OUT], mybir.dt.float32, kind="ExternalOutput")
    rmask = nc.inline_tensor(_build_masks(), name="rmask")

    dve = nc.vector
    gps = nc.gpsimd if USE_GPSIMD else nc.vector
    act = nc.scalar

    with TileContext(nc) as tc:
        with tc.tile_pool(name="pp", bufs=1) as pers, \
             tc.tile_pool(name="tp", bufs=2) as tP, \
             tc.tile_pool(name="tq", bufs=2) as tQ, \
             tc.tile_pool(name="tr", bufs=3) as tR, \
             tc.tile_pool(name="to", bufs=2) as tPo, \
             tc.tile_pool(name="tqo", bufs=2) as tQo, \
             tc.tile_pool(name="tro", bufs=2) as tRo, \
             tc.tile_pool(name="sg", bufs=1) as sg:

            # persistent tiles
            it_bufs = [pers.tile([128, 32, WIN], mybir.dt.float32, tag=f"in{i}", name=f"itile{i}")
                       for i in range(2)]
            ewt = pers.tile([128, 60, EW], DT, tag="ewt")
            owt = pers.tile([128, 60, EW], DT, tag="owt")
            ee = pers.tile([128, 28, EW], DT, tag="ee")
            oo = pers.tile([128, 28, EW], DT, tag="oo")
            s4t = pers.tile([128, 25, 121], DT, tag="s4")
            s4o = pers.tile([128, 22, 121], DT, tag="s4o")
            mask_t = pers.tile([128, 4, 16, OW], mybir.dt.uint8, tag="mk")
            out_t = pers.tile([128, 16, OW], mybir.dt.float32, tag="out")

            # pad columns of the input tiles: written once, never DMA'd over
            for itile in it_bufs:
                nc.gpsimd.memset(itile[0:64, :, 0:13], NEG)
                nc.gpsimd.memset(itile[64:128, :, 237:WIN], NEG)

            mx = mybir.AluOpType.max

            for it in range(15):
                y0 = max(0, 16 * it - 8)
                y1 = min(OUT, 16 * it + 8)
                H = y1 - y0
                J0 = 16 * it - 14          # EE/OO tile base row
                itile = it_bufs[it % 2]
                # which rings touch this row band (|dy| < R)
                dy_min = min(abs(yy - OUT // 2) for yy in range(y0, y1))
                ring = [dy_min * dy_min < r2 for r2 in R2]  # r1..r4 present

                # ---- input DMA ----
                if it < 14:
                    r0 = 32 * it
                    nc.sync.dma_start(itile[0:64, :, 13:WIN],
                                      x[:, r0:r0 + 32, 0:237])
                    nc.sync.dma_start(itile[64:128, :, 1:237],
                                      x[:, r0:r0 + 32, 212:448])

                # ---- Ew/Ow rolling tiles (base x-row X0 = 32it-28) ----
                if it == 0:
                    nc.gpsimd.memset(ewt[:, 0:28, :], NEG)
                    nc.gpsimd.memset(owt[:, 0:28, :], NEG)
                else:
                    act.copy(ewt[:, 0:28, :], ewt[:, 32:60, :])
                    act.copy(owt[:, 0:28, :], owt[:, 32:60, :])
                if it < 14:
                    # Ew[e] = max(T[2e+1], T[2e+2]); Ow[e] = max(T[2e+2], T[2e+3])
                    dve.tensor_tensor(ewt[:, 28:60, :],
                                      itile[:, :, 1:249:2],
                                      itile[:, :, 2:250:2], mx)
                    gps.tensor_tensor(owt[:, 28:60, :],
                                      itile[:, :, 2:250:2],
                                      itile[:, :, 3:250:2], mx)
                else:
                    nc.gpsimd.memset(ewt[:, 28:60, :], NEG)
                    nc.gpsimd.memset(owt[:, 28:60, :], NEG)

                # ---- EE / OO (fresh, 28 rows, base J0) ----
                if it == 0:
                    nc.gpsimd.memset(ee[:, 0:14, :], NEG)
                    nc.gpsimd.memset(oo[:, 0:14, :], NEG)
                    rlo = 14
                else:
                    rlo = 0
                need_oo = ring[1] or ring[3]
                dve.tensor_tensor(ee[:, rlo:28, :],
                                  ewt[:, 2 * rlo:56:2, :],
                                  ewt[:, 2 * rlo + 1:57:2, :], mx)
                # OO rows read Ow slots (2r+1, 2r+2)
                if need_oo:
                    gps.tensor_tensor(oo[:, rlo:28, :],
                                      owt[:, 2 * rlo + 1:57:2, :],
                                      owt[:, 2 * rlo + 2:58:2, :], mx)

                # ---- helpers ----
                def rows(tile_base, lo, hi):
                    return lo - tile_base, hi - tile_base

                def rcomb(dst, dst_base, src, src_base, jlo, jhi, d, w):
                    a, b = rows(src_base, jlo, jhi)
                    o0, o1 = rows(dst_base, jlo, jhi)
                    dve.tensor_tensor(dst[:, o0:o1, 0:w],
                                      src[:, a:b, 0:w],
                                      src[:, a + d:b + d, 0:w], mx)

                def ccomb(dst, src, nrows, d, w):
                    dve.tensor_tensor(dst[:, 0:nrows, 0:w],
                                      src[:, 0:nrows, 0:w],
                                      src[:, 0:nrows, d:d + w], mx)

                # ---- EE-side pyramid ----
                a2 = tP.tile([128, 27, EW], DT, tag="p0")
                rcomb(a2, y0 - 6, ee, J0, y0 - 6, y1 + 5, 1, EW)
                s2 = tQ.tile([128, 27, 123], DT, tag="q0")
                ccomb(s2, a2, H + 11, 1, 123)
                a4 = tP.tile([128, 27, 123], DT, tag="p0")
                rcomb(a4, y0 - 6, s2, y0 - 6, y0 - 6, y1 + 3, 2, 123)
                ccomb(s4t, a4, H + 9, 2, 121)
                a8 = tP.tile([128, 27, 121], DT, tag="p0")
                rcomb(a8, y0 - 6, s4t, y0 - 6, y0 - 6, y1 - 1, 4, 121)
                if ring[2]:
                    u = tR.tile([128, 16, 121], DT, tag="r0")
                    rcomb(u, y0 - 3, s4t, y0 - 6, y0 - 3, y1 - 3, 3, 121)
                    s7 = tR.tile([128, 16, 118], DT, tag="r0")
                    ccomb(s7, u, H, 3, 118)
                s8 = tQ.tile([128, 27, 117], DT, tag="q0")
                ccomb(s8, a8, H + 5, 4, 117)
                v = tR.tile([128, 16, 117], DT, tag="r0")
                rcomb(v, y0 - 6, s8, y0 - 6, y0 - 6, y1 - 6, 5, 117)
                s13 = tR.tile([128, 16, 112], DT, tag="r13", bufs=1)
                ccomb(s13, v, H, 5, 112)

                # ---- OO-side pyramid ----
                if need_oo:
                    a2o = tPo.tile([128, 24, EW], DT, tag="po")
                    rcomb(a2o, y0 - 5, oo, J0, y0 - 5, y1 + 3, 1, EW)
                    s2o = tQo.tile([128, 24, 123], DT, tag="qo")
                    ccomb(s2o, a2o, H + 8, 1, 123)
                    a4o = tPo.tile([128, 24, 123], DT, tag="po")
                    rcomb(a4o, y0 - 5, s2o, y0 - 5, y0 - 5, y1 + 1, 2, 123)
                    ccomb(s4o, a4o, H + 6, 2, 121)
                if ring[3]:
                    a8o = tPo.tile([128, 24, 121], DT, tag="po")
                    rcomb(a8o, y0 - 5, s4o, y0 - 5, y0 - 5, y1 - 3, 4, 121)
                    s8o = tQo.tile([128, 24, 117], DT, tag="qo")
                    ccomb(s8o, a8o, H + 2, 4, 117)
                    w = tRo.tile([128, 16, 117], DT, tag="ro")
                    rcomb(w, y0 - 5, s8o, y0 - 5, y0 - 5, y1 - 5, 2, 117)
                    s10 = tRo.tile([128, 16, 115], DT, tag="ro")
                    ccomb(s10, w, H, 2, 115)

                # ---- masks DMA ----
                nc.sync.dma_start(mask_t[:, :, 0:H, :], rmask[:, :, y0:y1, :])

                # ---- blend: acc = S13[:, 0:H, 0:112] in place ----
                acc = s13[:, 0:H, 0:112]
                plist = []
                if ring[3]:
                    plist.append((0, s10[:, 0:H, 1:113]))
                if ring[2]:
                    plist.append((1, s7[:, 0:H, 3:115]))
                if ring[1]:
                    plist.append((2, s4o[:, 3:H + 3, 4:116]))
                if ring[0]:
                    plist.append((3, ee[:, y0 - J0:y1 - J0, 6:118]))
                for ridx, pr in plist:
                    dve.copy_predicated(acc, mask_t[:, ridx, 0:H, :], pr)

                # ---- cast + store ----
                act.copy(out_t[:, 0:H, :], acc)
                yv = y[:, y0:y1, :].rearrange("c h (w o) -> w c h o", o=OW)
                nc.sync.dma_start(yv[0], out_t[0:64, 0:H, :])
                nc.sync.dma_start(yv[1], out_t[64:128, 0:H, :])

    return nc


def _get_nc():
    if "nc" not in _CACHED:
        nc = bass.Bass()
        _emit_kernel(nc)
        split_multi_waits(nc)
        _CACHED["nc"] = nc
    return _CACHED["nc"]


def kernel(x: np.ndarray) -> np.ndarray:
    nc = _get_nc()
    in_maps = [{"x": np.ascontiguousarray(x[b], dtype=np.float32)}
               for b in range(B)]
    res = run_bass_kernel_spmd(nc, in_maps, core_ids=list(range(B)))
    return np.stack([r["y"] for r in res.results]).astype(np.float32)



# revision 4
# speedup vs baseline: 3.5383x; 3.5383x over previous
"""Trainium2 Bass kernel for CenterDependentPool2D.

Input  x: (8, 64, 448, 448) fp32  ->  Output: (8, 64, 224, 224) fp32.

Strategy (per core = one batch element, 64 channels):
  - Partition p = c + 64*wg. Position w in [0,112): wg0 -> out col w,
    wg1 -> out col 223-w (wg1 input is column-MIRRORED on the host via a
    second DRAM tensor xr, so both wgs see "outer edge at w=0, center at
    w=111" and every ring interval is a contiguous [lo,hi) slice).
  - All five ring windows (k in {2,8,14,20,26}, stride 2, reflect pad)
    decompose over pair-max arrays Ew[e]=max(x[2e],x[2e+1]) (stored at
    j=e+6) and Ow (odd pairs), pooled vertically to EE/OO, then a shared
    shifted-max pyramid per ring. Reflect padding == window clipping here,
    so out-of-range leaves are -BIG fills.
  - NEW vs v0: every pyramid op is restricted to the column interval where
    its ring can win (ring r lives in the annulus R_{r-1} < d < R_r, which
    in mirrored position space is one interval per band). The blend is
    per-band: the outer ring writes acc directly, inner rings do one plain
    segment copy (fp16 4x) plus one narrow copy_predicated boundary strip.
  - 8 bands x 28 rows. Rolling Ew/Ow raw-pair tiles (roll on scalar).
    fp16 pipeline after the first max; final cast on scalar engine.
"""

import math
import numpy as np

import concourse.bass as bass
import concourse.mybir as mybir
from concourse.tile import TileContext
from concourse.bass_utils import run_bass_kernel_spmd

# ---------------- problem constants ----------------
B, C, IN, OUT = 8, 64, 448, 224
BH = 28                   # out rows per band
NB = OUT // BH            # 8 bands
RADII = [60, 75, 90, 105]
NEG = -30000.0
EW = 124                  # E/O array width (j = position e + 6)
RE = 2 * BH + 24          # 80 raw rows resident per band
ITC = 16                  # itile chunk rows
DT = mybir.dt.float16

_CACHED = {}


# ---------------- geometry ----------------
def band_geometry():
    bands = []
    for i in range(NB):
        y0 = i * BH
        rings = []
        for R in RADII:
            bmin, bmax, present = 112, 0, False
            for yy in range(y0, y0 + BH):
                dy = abs(yy - 112)
                if dy < R:
                    present = True
                    s = math.sqrt(R * R - dy * dy)
                    bmin = min(bmin, 111 - s)
                    bmax = max(bmax, 112 - s)
                else:
                    bmax = 112
            if not present:
                rings.append(None)
            else:
                rings.append((max(0, math.floor(bmin)),
                              min(112, math.floor(bmax) + 1)))
        b1, b2, b3, b4 = rings
        hi = lambda r: r[1] if r else 112
        I5 = (0, hi(b4))
        I4 = (b4[0], hi(b3)) if b4 else None
        I3 = (b3[0], hi(b2)) if b3 else None
        I2 = (b2[0], hi(b1)) if b2 else None
        I1 = (b1[0], 112) if b1 else None
        bands.append(dict(y0=y0, strips=[b1, b2, b3, b4],
                          I=[I1, I2, I3, I4, I5]))
    return bands


def merge_ivs(ivs, gap=14):
    ivs = sorted([list(v) for v in ivs if v is not None])
    out = []
    for iv in ivs:
        if out and iv[0] <= out[-1][1] + gap:
            out[-1][1] = max(out[-1][1], iv[1])
        else:
            out.append(iv)
    return [tuple(v) for v in out]


BANDS = band_geometry()


def build_masks():
    """Packed per-band boundary-strip masks [128, 28, TOTW] u8; per band the
    ring blocks (r4,r3,r2,r1 order) are contiguous so one DMA per band."""
    yy = np.arange(OUT)
    blocks, offs, off = [], {}, 0
    for bi, bd in enumerate(BANDS):
        for ri in (3, 2, 1, 0):
            st = bd['strips'][ri]
            if st is None:
                continue
            lo, hi = st
            w = hi - lo
            R2 = RADII[ri] ** 2
            rows = yy[bd['y0']:bd['y0'] + BH]
            dy2 = (rows - 112) ** 2                       # [28]
            wpos = np.arange(lo, hi)
            m = np.zeros((128, BH, w), np.uint8)
            dx0 = (112 - wpos) ** 2
            dx1 = (111 - wpos) ** 2
            m[0:64] = (dy2[None, :, None] + dx0[None, None, :] < R2)
            m[64:128] = (dy2[None, :, None] + dx1[None, None, :] < R2)
            offs[(bi, ri)] = (off, w)
            blocks.append(m)
            off += w
    return np.concatenate(blocks, axis=2), offs


MASKS, MOFFS = build_masks()
MTOT = MASKS.shape[2]
MBAND = {}               # band -> (off, width) of its contiguous mask block
for bi in range(NB):
    pieces = [MOFFS[(bi, ri)] for ri in (3, 2, 1, 0) if (bi, ri) in MOFFS]
    MBAND[bi] = (pieces[0][0], sum(w for _, w in pieces))
MW = max(w for _, w in MBAND.values())


def split_multi_waits(nc):
    """walrus CoreV3Gen accepts at most 1 sync-wait per instruction; peel
    extras onto preceding NoOps."""
    n = 0
    for fn in nc.m.functions:
        for bb in fn.blocks:
            insts = list(bb.instructions)
            out = []
            for ins in insts:
                si = getattr(ins, "sync_info", None)
                if si is not None and len(si.on_wait) > 1:
                    waits = list(si.on_wait)
                    for k, w in enumerate(waits[:-1]):
                        nop = mybir.InstNoOp(
                            name=f"{ins.name}-waitsplit{k}",
                            engine=ins.engine, ins=[], outs=[])
                        nop.sync_info = mybir.SyncInfo(on_wait=[w], on_update=[])
                        out.append(nop)
                        n += 1
                    ins.sync_info = mybir.SyncInfo(
                        on_wait=[waits[-1]], on_update=list(si.on_update))
                out.append(ins)
            if n:
                bb.instructions = out
    return n


def _emit_kernel(nc: bass.Bass):
    x = nc.dram_tensor("x", [C, IN, IN], mybir.dt.float32, kind="ExternalInput")
    xr = nc.dram_tensor("xr", [C, IN, IN], mybir.dt.float32, kind="ExternalInput")
    y0d = nc.dram_tensor("y0d", [C, OUT, 112], mybir.dt.float32, kind="ExternalOutput")
    y1d = nc.dram_tensor("y1d", [C, OUT, 112], mybir.dt.float32, kind="ExternalOutput")
    rmask = nc.inline_tensor(MASKS, name="rmask")

    dve = nc.vector
    act = nc.scalar
    mx = mybir.AluOpType.max

    with TileContext(nc) as tc:
        with tc.tile_pool(name="pp", bufs=1) as pers, \
             tc.tile_pool(name="ts2", bufs=1) as tS2, \
             tc.tile_pool(name="ts4", bufs=1) as tS4, \
             tc.tile_pool(name="tmp", bufs=1) as tT, \
             tc.tile_pool(name="ts8", bufs=1) as tS8, \
             tc.tile_pool(name="fin", bufs=1) as tF, \
             tc.tile_pool(name="tac", bufs=2) as tA, \
             tc.tile_pool(name="tou", bufs=1) as tO, \
             tc.tile_pool(name="tmk", bufs=2) as tM:

            it_bufs = [pers.tile([128, ITC, 250], mybir.dt.float32,
                                 tag=f"in{i}", name=f"itile{i}") for i in range(2)]
            ewt = pers.tile([128, RE, EW], DT, tag="ewt")
            owt = pers.tile([128, RE, EW], DT, tag="owt")
            ee = pers.tile([128, 40, EW], DT, tag="ee")
            oo = pers.tile([128, 38, EW], DT, tag="oo")

            # pad columns of itiles (never DMA'd over)
            for itile in it_bufs:
                nc.gpsimd.memset(itile[:, :, 0:13], NEG)
                nc.gpsimd.memset(itile[64:128, :, 249:250], NEG)

            chunk_idx = 0
            for bi, bd in enumerate(BANDS):
                y0 = bd['y0']
                I1, I2, I3, I4, I5 = bd['I']
                b1, b2, b3, b4 = bd['strips']
                g0 = 2 * y0 - 12

                # ---- roll ewt/owt (reads rows 56:80 before fresh overwrite) ----
                if bi > 0:
                    act.copy(ewt[:, 0:24, :], ewt[:, 56:80, :])
                    act.copy(owt[:, 0:24, :], owt[:, 56:80, :])

                # ---- fresh pair rows via itile chunks ----
                klo = 12 if bi == 0 else 24
                khi = min(RE, IN - g0)
                k = klo
                while k < khi:
                    n = min(ITC, khi - k)
                    itile = it_bufs[chunk_idx % 2]
                    chunk_idx += 1
                    ga = g0 + k
                    nc.sync.dma_start(itile[0:64, 0:n, 13:250],
                                      x[:, ga:ga + n, 0:237])
                    nc.sync.dma_start(itile[64:128, 0:n, 13:249],
                                      xr[:, ga:ga + n, 0:236])
                    dve.tensor_tensor(ewt[:, k:k + n, :],
                                      itile[:, 0:n, 1:249:2],
                                      itile[:, 0:n, 2:250:2], mx)
                    dve.tensor_tensor(owt[:, k:k + n, :],
                                      itile[:, 0:n, 2:250:2],
                                      itile[:, 0:n, 3:250:2], mx)
                    k += n
                if khi < RE:
                    nc.gpsimd.memset(ewt[:, khi:RE, :], NEG)
                    nc.gpsimd.memset(owt[:, khi:RE, :], NEG)

                # ---- masks DMA for this band ----
                moff, mw = MBAND[bi]
                mask_t = tM.tile([128, BH, MW], mybir.dt.uint8, tag="mk")
                nc.sync.dma_start(mask_t[:, :, 0:mw], rmask[:, :, moff:moff + mw])

                # ---- ee / oo ----
                dve.tensor_tensor(ee[:, 0:40, :], ewt[:, 0:80:2, :],
                                  ewt[:, 1:80:2, :], mx)
                need_oo = I4 or I2
                if need_oo:
                    dve.tensor_tensor(oo[:, 0:38, :], owt[:, 3:79:2, :],
                                      owt[:, 4:80:2, :], mx)

                # ---- acc ----
                acc = tA.tile([128, BH, 112], DT, tag="acc")

                # ================= EE chain =================
                l5, h5 = I5
                iv2 = [(0, min(EW, h5 + 12))]
                iv4 = [(0, min(EW, h5 + 11))]
                if I3:
                    l3, h3 = I3
                    iv2.append((l3 + 3, min(EW, h3 + 9)))
                    iv4.append((l3 + 3, min(EW, h3 + 8)))
                iv2 = merge_ivs(iv2)
                iv4 = merge_ivs(iv4)

                s2 = tS2.tile([128, 39, EW], DT, tag="s2")
                for lo, hi in iv2:
                    a2 = tT.tile([128, 39, EW], DT, tag="tmp")
                    dve.tensor_tensor(a2[:, :, lo:hi], ee[:, 0:39, lo:hi],
                                      ee[:, 1:40, lo:hi], mx)
                    dve.tensor_tensor(s2[:, :, lo:hi - 1], a2[:, :, lo:hi - 1],
                                      a2[:, :, lo + 1:hi], mx)
                s4 = tS4.tile([128, 37, EW], DT, tag="s4")
                for lo, hi in iv4:
                    a4 = tT.tile([128, 37, EW], DT, tag="tmp")
                    dve.tensor_tensor(a4[:, :, lo:hi], s2[:, 0:37, lo:hi],
                                      s2[:, 2:39, lo:hi], mx)
                    dve.tensor_tensor(s4[:, :, lo:hi - 2], a4[:, :, lo:hi - 2],
                                      a4[:, :, lo + 2:hi], mx)
                # P5 finals
                w8 = min(EW, h5 + 9)
                a8 = tT.tile([128, 33, EW], DT, tag="tmp")
                dve.tensor_tensor(a8[:, :, 0:w8], s4[:, 0:33, 0:w8],
                                  s4[:, 4:37, 0:w8], mx)
                s8 = tS8.tile([128, 33, EW], DT, tag="s8")
                dve.tensor_tensor(s8[:, :, 0:w8 - 4], a8[:, :, 0:w8 - 4],
                                  a8[:, :, 4:w8], mx)
                v13 = tF.tile([128, BH, EW], DT, tag="fin")
                dve.tensor_tensor(v13[:, :, 0:w8 - 4], s8[:, 0:28, 0:w8 - 4],
                                  s8[:, 5:33, 0:w8 - 4], mx)
                # s13 writes acc[0:h5) directly
                dve.tensor_tensor(acc[:, :, 0:h5], v13[:, :, 0:h5],
                                  v13[:, :, 5:h5 + 5], mx)
                if I3:
                    u = tT.tile([128, BH, EW], DT, tag="tmp")
                    dve.tensor_tensor(u[:, :, l3 + 3:h3 + 6],
                                      s4[:, 3:31, l3 + 3:h3 + 6],
                                      s4[:, 6:34, l3 + 3:h3 + 6], mx)
                    s7b = tF.tile([128, BH, EW], DT, tag="s7b")
                    dve.tensor_tensor(s7b[:, :, 0:h3 - l3],
                                      u[:, :, l3 + 3:h3 + 3],
                                      u[:, :, l3 + 6:h3 + 6], mx)

                # ================= OO chain =================
                if need_oo:
                    ivo2, ivo4 = [], []
                    if I4:
                        l4, h4 = I4
                        ivo2.append((l4 + 1, min(EW, h4 + 10)))
                        ivo4.append((l4 + 1, min(EW, h4 + 9)))
                    if I2:
                        l2, h2 = I2
                        ivo2.append((l2 + 4, min(EW, h2 + 7)))
                        ivo4.append((l2 + 4, min(EW, h2 + 6)))
                    ivo2 = merge_ivs(ivo2)
                    ivo4 = merge_ivs(ivo4)
                    s2o = tS2.tile([128, 37, EW], DT, tag="s2o")
                    for lo, hi in ivo2:
                        a2o = tT.tile([128, 37, EW], DT, tag="tmp")
                        dve.tensor_tensor(a2o[:, :, lo:hi], oo[:, 0:37, lo:hi],
                                          oo[:, 1:38, lo:hi], mx)
                        dve.tensor_tensor(s2o[:, :, lo:hi - 1],
                                          a2o[:, :, lo:hi - 1],
                                          a2o[:, :, lo + 1:hi], mx)
                    s4o = tS4.tile([128, 35, EW], DT, tag="s4o")
                    for lo, hi in ivo4:
                        a4o = tT.tile([128, 35, EW], DT, tag="tmp")
                        dve.tensor_tensor(a4o[:, :, lo:hi], s2o[:, 0:35, lo:hi],
                                          s2o[:, 2:37, lo:hi], mx)
                        dve.tensor_tensor(s4o[:, :, lo:hi - 2],
                                          a4o[:, :, lo:hi - 2],
                                          a4o[:, :, lo + 2:hi], mx)
                    if I4:
                        w8o = min(EW, h4 + 7)
                        a8o = tT.tile([128, 31, EW], DT, tag="tmp")
                        dve.tensor_tensor(a8o[:, :, l4 + 1:w8o],
                                          s4o[:, 0:31, l4 + 1:w8o],
                                          s4o[:, 4:35, l4 + 1:w8o], mx)
                        s8o = tS8.tile([128, 31, EW], DT, tag="s8")
                        dve.tensor_tensor(s8o[:, :, 0:w8o - l4 - 5],
                                          a8o[:, :, l4 + 1:w8o - 4],
                                          a8o[:, :, l4 + 5:w8o], mx)
                        # s8o col q == j = l4+1+q, valid q in [0, h4+2-l4)
                        v10 = tF.tile([128, BH, EW], DT, tag="fin")
                        nq = h4 + 2 - l4
                        dve.tensor_tensor(v10[:, :, 0:nq], s8o[:, 0:28, 0:nq],
                                          s8o[:, 2:30, 0:nq], mx)
                        p4v = tF.tile([128, BH, EW], DT, tag="p4v")
                        dve.tensor_tensor(p4v[:, :, 0:h4 - l4],
                                          v10[:, :, 0:h4 - l4],
                                          v10[:, :, 2:h4 - l4 + 2], mx)

                # ================= blend =================
                hi_of = lambda r: r[1] if r else 112
                if b4:
                    lo, hi = b4
                    p3m = hi_of(b3)
                    if p3m > hi:
                        dve.tensor_copy(acc[:, :, hi:p3m],
                                        p4v[:, :, hi - l4:p3m - l4])
                    off, w = MOFFS[(bi, 3)]
                    dve.copy_predicated(acc[:, :, lo:hi],
                                        mask_t[:, :, off - moff:off - moff + w],
                                        p4v[:, :, lo - l4:hi - l4])
                if b3:
                    lo, hi = b3
                    p2m = hi_of(b2)
                    if p2m > hi:
                        dve.tensor_copy(acc[:, :, hi:p2m],
                                        s7b[:, :, hi - l3:p2m - l3])
                    off, w = MOFFS[(bi, 2)]
                    dve.copy_predicated(acc[:, :, lo:hi],
                                        mask_t[:, :, off - moff:off - moff + w],
                                        s7b[:, :, lo - l3:hi - l3])
                if b2:
                    lo, hi = b2
                    p1m = hi_of(b1)
                    if p1m > hi:
                        dve.tensor_copy(acc[:, :, hi:p1m],
                                        s4o[:, 3:31, hi + 4:p1m + 4])
                    off, w = MOFFS[(bi, 1)]
                    dve.copy_predicated(acc[:, :, lo:hi],
                                        mask_t[:, :, off - moff:off - moff + w],
                                        s4o[:, 3:31, lo + 4:hi + 4])
                if b1:
                    lo, hi = b1
                    if 112 > hi:
                        dve.tensor_copy(acc[:, :, hi:112],
                                        ee[:, 6:34, hi + 6:118])
                    off, w = MOFFS[(bi, 0)]
                    dve.copy_predicated(acc[:, :, lo:hi],
                                        mask_t[:, :, off - moff:off - moff + w],
                                        ee[:, 6:34, lo + 6:hi + 6])

                # ---- cast + store ----
                out_t = tO.tile([128, BH, 112], mybir.dt.float32, tag="out")
                act.copy(out_t[:, :, :], acc[:, :, :])
                nc.sync.dma_start(y0d[:, y0:y0 + BH, :], out_t[0:64, :, :])
                nc.sync.dma_start(y1d[:, y0:y0 + BH, :], out_t[64:128, :, :])

    return nc


def _get_nc():
    if "nc" not in _CACHED:
        nc = bass.Bass()
        _emit_kernel(nc)
        split_multi_waits(nc)
        _CACHED["nc"] = nc
    return _CACHED["nc"]


def _in_maps(x):
    maps = []
    for b in range(B):
        xb = np.ascontiguousarray(x[b], dtype=np.float32)
        xrb = np.ascontiguousarray(xb[:, :, ::-1])
        maps.append({"x": xb, "xr": xrb})
    return maps


def kernel(x: np.ndarray) -> np.ndarray:
    nc = _get_nc()
    res = run_bass_kernel_spmd(nc, _in_maps(x), core_ids=list(range(B)))
    out = np.empty((B, C, OUT, OUT), np.float32)
    for b, r in enumerate(res.results):
        out[b, :, :, 0:112] = r["y0d"]
        out[b, :, :, 112:224] = r["y1d"][:, :, ::-1]
    return out


# revision 7
# speedup vs baseline: 3.7224x; 1.0520x over previous
"""Trainium2 Bass kernel for CenterDependentPool2D.

Input  x: (8, 64, 448, 448) fp32  ->  Output: (8, 64, 224, 224) fp32.

Strategy (per core = one batch element, 64 channels):
  - Partition p = c + 64*wg. Position w in [0,112): wg0 -> out col w,
    wg1 -> out col 223-w (wg1 input is column-MIRRORED on the host via a
    second DRAM tensor xr, so both wgs see "outer edge at w=0, center at
    w=111" and every ring interval is a contiguous [lo,hi) slice).
  - All five ring windows (k in {2,8,14,20,26}, stride 2, reflect pad)
    decompose over pair-max arrays Ew[e]=max(x[2e],x[2e+1]) (stored at
    j=e+6) and Ow (odd pairs), pooled vertically to EE/OO, then a shared
    shifted-max pyramid per ring. Reflect padding == window clipping here,
    so out-of-range leaves are -BIG fills.
  - NEW vs v0: every pyramid op is restricted to the column interval where
    its ring can win (ring r lives in the annulus R_{r-1} < d < R_r, which
    in mirrored position space is one interval per band). The blend is
    per-band: the outer ring writes acc directly, inner rings do one plain
    segment copy (fp16 4x) plus one narrow copy_predicated boundary strip.
  - 8 bands x 28 rows. Rolling Ew/Ow raw-pair tiles (roll on scalar).
    fp16 pipeline after the first max; final cast on scalar engine.
"""

import math
import numpy as np

import concourse.bass as bass
import concourse.mybir as mybir
from concourse.tile import TileContext
from concourse.bass_utils import run_bass_kernel_spmd

# ---------------- problem constants ----------------
B, C, IN, OUT = 8, 64, 448, 224
BH = 28                   # out rows per band
NB = OUT // BH            # 8 bands
RADII = [60, 75, 90, 105]
NEG = -30000.0
EW = 124                  # E/O array width (j = position e + 6)
RE = 2 * BH + 24          # 80 raw rows resident per band
ITC = 16                  # itile chunk rows
DT = mybir.dt.float16

_CACHED = {}


# ---------------- geometry ----------------
def band_geometry():
    bands = []
    for i in range(NB):
        y0 = i * BH
        rings = []
        for R in RADII:
            bmin, bmax, present = 112, 0, False
            for yy in range(y0, y0 + BH):
                dy = abs(yy - 112)
                if dy < R:
                    present = True
                    s = math.sqrt(R * R - dy * dy)
                    bmin = min(bmin, 111 - s)
                    bmax = max(bmax, 112 - s)
                else:
                    bmax = 112
            if not present:
                rings.append(None)
            else:
                rings.append((max(0, math.floor(bmin)),
                              min(112, math.floor(bmax) + 1)))
        b1, b2, b3, b4 = rings
        hi = lambda r: r[1] if r else 112
        I5 = (0, hi(b4))
        I4 = (b4[0], hi(b3)) if b4 else None
        I3 = (b3[0], hi(b2)) if b3 else None
        I2 = (b2[0], hi(b1)) if b2 else None
        I1 = (b1[0], 112) if b1 else None
        bands.append(dict(y0=y0, strips=[b1, b2, b3, b4],
                          I=[I1, I2, I3, I4, I5]))
    return bands


def merge_ivs(ivs, gap=14):
    ivs = sorted([list(v) for v in ivs if v is not None])
    out = []
    for iv in ivs:
        if out and iv[0] <= out[-1][1] + gap:
            out[-1][1] = max(out[-1][1], iv[1])
        else:
            out.append(iv)
    return [tuple(v) for v in out]


BANDS = band_geometry()


def build_masks():
    """Packed per-band boundary-strip masks [128, 28, TOTW] u8; per band the
    ring blocks (r4,r3,r2,r1 order) are contiguous so one DMA per band."""
    yy = np.arange(OUT)
    blocks, offs, off = [], {}, 0
    for bi, bd in enumerate(BANDS):
        for ri in (3, 2, 1, 0):
            st = bd['strips'][ri]
            if st is None:
                continue
            lo, hi = st
            w = hi - lo
            R2 = RADII[ri] ** 2
            rows = yy[bd['y0']:bd['y0'] + BH]
            dy2 = (rows - 112) ** 2                       # [28]
            wpos = np.arange(lo, hi)
            m = np.zeros((128, BH, w), np.uint8)
            dx0 = (112 - wpos) ** 2
            dx1 = (111 - wpos) ** 2
            m[0:64] = (dy2[None, :, None] + dx0[None, None, :] < R2)
            m[64:128] = (dy2[None, :, None] + dx1[None, None, :] < R2)
            offs[(bi, ri)] = (off, w)
            blocks.append(m)
            off += w
    return np.concatenate(blocks, axis=2), offs


MASKS, MOFFS = build_masks()
MTOT = MASKS.shape[2]
MBAND = {}               # band -> (off, width) of its contiguous mask block
for bi in range(NB):
    pieces = [MOFFS[(bi, ri)] for ri in (3, 2, 1, 0) if (bi, ri) in MOFFS]
    MBAND[bi] = (pieces[0][0], sum(w for _, w in pieces))
MW = max(w for _, w in MBAND.values())


def split_multi_waits(nc):
    """walrus CoreV3Gen accepts at most 1 sync-wait per instruction; peel
    extras onto preceding NoOps."""
    n = 0
    for fn in nc.m.functions:
        for bb in fn.blocks:
            insts = list(bb.instructions)
            out = []
            for ins in insts:
                si = getattr(ins, "sync_info", None)
                if si is not None and len(si.on_wait) > 1:
                    waits = list(si.on_wait)
                    for k, w in enumerate(waits[:-1]):
                        nop = mybir.InstNoOp(
                            name=f"{ins.name}-waitsplit{k}",
                            engine=ins.engine, ins=[], outs=[])
                        nop.sync_info = mybir.SyncInfo(on_wait=[w], on_update=[])
                        out.append(nop)
                        n += 1
                    ins.sync_info = mybir.SyncInfo(
                        on_wait=[waits[-1]], on_update=list(si.on_update))
                out.append(ins)
            if n:
                bb.instructions = out
    return n


def _emit_kernel(nc: bass.Bass):
    x = nc.dram_tensor("x", [C, IN, IN], mybir.dt.float32, kind="ExternalInput")
    xr = nc.dram_tensor("xr", [C, IN, IN], mybir.dt.float32, kind="ExternalInput")
    y0d = nc.dram_tensor("y0d", [C, OUT, 112], mybir.dt.float32, kind="ExternalOutput")
    y1d = nc.dram_tensor("y1d", [C, OUT, 112], mybir.dt.float32, kind="ExternalOutput")
    rmask = nc.inline_tensor(MASKS, name="rmask")

    dve = nc.vector
    act = nc.scalar
    mx = mybir.AluOpType.max

    with TileContext(nc) as tc:
        with tc.tile_pool(name="pp", bufs=1) as pers, \
             tc.tile_pool(name="ts2", bufs=1) as tS2, \
             tc.tile_pool(name="ts4", bufs=1) as tS4, \
             tc.tile_pool(name="tmp", bufs=1) as tT, \
             tc.tile_pool(name="ts8", bufs=1) as tS8, \
             tc.tile_pool(name="fin", bufs=1) as tF, \
             tc.tile_pool(name="tac", bufs=1) as tA, \
             tc.tile_pool(name="tou", bufs=1) as tO, \
             tc.tile_pool(name="tmk", bufs=2) as tM, \
             tc.tile_pool(name="tit", bufs=3) as tIT:

            ewt = pers.tile([128, RE, EW], DT, tag="ewt")
            owt = pers.tile([128, RE, EW], DT, tag="owt")
            ee = pers.tile([128, 40, EW], DT, tag="ee")
            oo = pers.tile([128, 38, EW], DT, tag="oo")

            # initial NEG fill of the top reflect-pad rows
            nc.gpsimd.memset(ewt[:, 0:12, :], NEG)
            nc.gpsimd.memset(owt[:, 0:12, :], NEG)

            # per-band oo/owt column intervals (chains only need these cols)
            def oo_cols(bj):
                Ii = BANDS[bj]['I']
                ivs = []
                if Ii[3]:
                    ivs.append((Ii[3][0] + 1, min(EW, Ii[3][1] + 10)))
                if Ii[1]:
                    ivs.append((Ii[1][0] + 4, min(EW, Ii[1][1] + 7)))
                return merge_ivs(ivs)

            OC = [oo_cols(bj) for bj in range(NB)]
            OCU = [merge_ivs(OC[bj] + (OC[bj + 1] if bj + 1 < NB else []))
                   for bj in range(NB)]

            chunk_state = {"idx": 0}
            pend = {}          # band -> list of (itile, k, n)
            mtiles = {}        # band -> mask tile

            def chunk_dma(ga, n):
                """Allocate a fresh pool itile (rotation waits for the previous
                tenant's readers -> no DMA-clobber race), set its NEG pads on
                gpsimd, then DMA the chunk rows."""
                itile = tIT.tile([128, ITC, 250], mybir.dt.float32, tag="it")
                nc.gpsimd.memset(itile[:, 0:n, 0:13], NEG)
                nc.gpsimd.memset(itile[64:128, 0:n, 249:250], NEG)
                nc.sync.dma_start(itile[0:64, 0:n, 13:250],
                                  x[:, ga:ga + n, 0:237])
                nc.sync.dma_start(itile[64:128, 0:n, 13:249],
                                  xr[:, ga:ga + n, 0:236])
                return itile

            def emit_dma(bj):
                """Issue the input DMAs + mask DMA for band bj."""
                gg0 = 2 * BANDS[bj]['y0'] - 12
                klo = 12 if bj == 0 else 24
                khi = min(RE, IN - gg0)
                lst = []
                k = klo
                while k < khi:
                    n = min(ITC, khi - k)
                    lst.append((chunk_dma(gg0 + k, n), k, n))
                    k += n
                pend[bj] = lst
                moff, mw = MBAND[bj]
                mask_t = tM.tile([128, BH, MW], mybir.dt.uint8, tag="mk")
                nc.sync.dma_start(mask_t[:, :, 0:mw], rmask[:, :, moff:moff + mw])
                mtiles[bj] = mask_t

            emit_dma(0)
            for bi, bd in enumerate(BANDS):
                y0 = bd['y0']
                I1, I2, I3, I4, I5 = bd['I']
                b1, b2, b3, b4 = bd['strips']
                g0 = 2 * y0 - 12
                khi = min(RE, IN - g0)

                # ---- fresh pair rows (input DMA'd during previous band) ----
                for itile, k, n in pend.pop(bi):
                    dve.tensor_tensor(ewt[:, k:k + n, :],
                                      itile[:, 0:n, 1:249:2],
                                      itile[:, 0:n, 2:250:2], mx)
                    for lo, hi in OCU[bi]:
                        dve.tensor_tensor(owt[:, k:k + n, lo:hi],
                                          itile[:, 0:n, 2 + 2 * lo:2 + 2 * hi:2],
                                          itile[:, 0:n, 3 + 2 * lo:3 + 2 * hi:2],
                                          mx)
                if khi < RE:
                    nc.gpsimd.memset(ewt[:, khi:RE, :], NEG)
                    nc.gpsimd.memset(owt[:, khi:RE, :], NEG)
                mask_t = mtiles.pop(bi)
                moff, mw = MBAND[bi]

                # ---- ee / oo ----
                dve.tensor_tensor(ee[:, 0:40, :], ewt[:, 0:80:2, :],
                                  ewt[:, 1:80:2, :], mx)
                need_oo = I4 or I2
                for lo, hi in OC[bi]:
                    dve.tensor_tensor(oo[:, 0:38, lo:hi], owt[:, 3:79:2, lo:hi],
                                      owt[:, 4:80:2, lo:hi], mx)

                # ---- prefetch band bi+1: roll (scalar) + input/mask DMA ----
                if bi + 1 < NB:
                    act.copy(ewt[:, 0:24, :], ewt[:, 56:80, :])
                    for lo, hi in OC[bi + 1]:
                        act.copy(owt[:, 0:24, lo:hi], owt[:, 56:80, lo:hi])
                    emit_dma(bi + 1)

                # ---- acc ----
                acc = tA.tile([128, BH, 112], DT, tag="acc")

                # ================= EE chain =================
                l5, h5 = I5
                iv2 = [(0, min(EW, h5 + 12))]
                iv4 = [(0, min(EW, h5 + 11))]
                if I3:
                    l3, h3 = I3
                    iv2.append((l3 + 3, min(EW, h3 + 9)))
                    iv4.append((l3 + 3, min(EW, h3 + 8)))
                iv2 = merge_ivs(iv2)
                iv4 = merge_ivs(iv4)

                s2 = tS2.tile([128, 39, EW], DT, tag="s2")
                for lo, hi in iv2:
                    a2 = tT.tile([128, 39, EW], DT, tag="tmp")
                    dve.tensor_tensor(a2[:, :, lo:hi], ee[:, 0:39, lo:hi],
                                      ee[:, 1:40, lo:hi], mx)
                    dve.tensor_tensor(s2[:, :, lo:hi - 1], a2[:, :, lo:hi - 1],
                                      a2[:, :, lo + 1:hi], mx)
                s4 = tS4.tile([128, 37, EW], DT, tag="s4")
                for lo, hi in iv4:
                    a4 = tT.tile([128, 37, EW], DT, tag="tmp")
                    dve.tensor_tensor(a4[:, :, lo:hi], s2[:, 0:37, lo:hi],
                                      s2[:, 2:39, lo:hi], mx)
                    dve.tensor_tensor(s4[:, :, lo:hi - 2], a4[:, :, lo:hi - 2],
                                      a4[:, :, lo + 2:hi], mx)
                # P5 finals
                w8 = min(EW, h5 + 9)
                a8 = tT.tile([128, 33, EW], DT, tag="tmp")
                dve.tensor_tensor(a8[:, :, 0:w8], s4[:, 0:33, 0:w8],
                                  s4[:, 4:37, 0:w8], mx)
                s8 = tS8.tile([128, 33, EW], DT, tag="s8")
                dve.tensor_tensor(s8[:, :, 0:w8 - 4], a8[:, :, 0:w8 - 4],
                                  a8[:, :, 4:w8], mx)
                v13 = tF.tile([128, BH, EW], DT, tag="fin")
                dve.tensor_tensor(v13[:, :, 0:w8 - 4], s8[:, 0:28, 0:w8 - 4],
                                  s8[:, 5:33, 0:w8 - 4], mx)
                # s13 writes acc[0:h5) directly
                dve.tensor_tensor(acc[:, :, 0:h5], v13[:, :, 0:h5],
                                  v13[:, :, 5:h5 + 5], mx)
                if I3:
                    u = tT.tile([128, BH, EW], DT, tag="tmp")
                    dve.tensor_tensor(u[:, :, l3 + 3:h3 + 6],
                                      s4[:, 3:31, l3 + 3:h3 + 6],
                                      s4[:, 6:34, l3 + 3:h3 + 6], mx)
                    s7b = tF.tile([128, BH, 80], DT, tag="s7b")
                    dve.tensor_tensor(s7b[:, :, 0:h3 - l3],
                                      u[:, :, l3 + 3:h3 + 3],
                                      u[:, :, l3 + 6:h3 + 6], mx)

                # ================= OO chain =================
                if need_oo:
                    ivo2, ivo4 = [], []
                    if I4:
                        l4, h4 = I4
                        ivo2.append((l4 + 1, min(EW, h4 + 10)))
                        ivo4.append((l4 + 1, min(EW, h4 + 9)))
                    if I2:
                        l2, h2 = I2
                        ivo2.append((l2 + 4, min(EW, h2 + 7)))
                        ivo4.append((l2 + 4, min(EW, h2 + 6)))
                    ivo2 = merge_ivs(ivo2)
                    ivo4 = merge_ivs(ivo4)
                    s2o = tS2.tile([128, 37, EW], DT, tag="s2o")
                    for lo, hi in ivo2:
                        a2o = tT.tile([128, 37, EW], DT, tag="tmp")
                        dve.tensor_tensor(a2o[:, :, lo:hi], oo[:, 0:37, lo:hi],
                                          oo[:, 1:38, lo:hi], mx)
                        dve.tensor_tensor(s2o[:, :, lo:hi - 1],
                                          a2o[:, :, lo:hi - 1],
                                          a2o[:, :, lo + 1:hi], mx)
                    s4o = tS4.tile([128, 35, EW], DT, tag="s4o")
                    for lo, hi in ivo4:
                        a4o = tT.tile([128, 35, EW], DT, tag="tmp")
                        dve.tensor_tensor(a4o[:, :, lo:hi], s2o[:, 0:35, lo:hi],
                                          s2o[:, 2:37, lo:hi], mx)
                        dve.tensor_tensor(s4o[:, :, lo:hi - 2],
                                          a4o[:, :, lo:hi - 2],
                                          a4o[:, :, lo + 2:hi], mx)
                    if I4:
                        w8o = min(EW, h4 + 7)
                        a8o = tT.tile([128, 31, EW], DT, tag="tmp")
                        dve.tensor_tensor(a8o[:, :, l4 + 1:w8o],
                                          s4o[:, 0:31, l4 + 1:w8o],
                                          s4o[:, 4:35, l4 + 1:w8o], mx)
                        s8o = tS8.tile([128, 31, EW], DT, tag="s8")
                        dve.tensor_tensor(s8o[:, :, 0:w8o - l4 - 5],
                                          a8o[:, :, l4 + 1:w8o - 4],
                                          a8o[:, :, l4 + 5:w8o], mx)
                        # s8o col q == j = l4+1+q, valid q in [0, h4+2-l4)
                        v10 = tF.tile([128, BH, EW], DT, tag="fin")
                        nq = h4 + 2 - l4
                        dve.tensor_tensor(v10[:, :, 0:nq], s8o[:, 0:28, 0:nq],
                                          s8o[:, 2:30, 0:nq], mx)
                        p4v = tF.tile([128, BH, 80], DT, tag="p4v")
                        dve.tensor_tensor(p4v[:, :, 0:h4 - l4],
                                          v10[:, :, 0:h4 - l4],
                                          v10[:, :, 2:h4 - l4 + 2], mx)

                # ================= blend =================
                hi_of = lambda r: r[1] if r else 112
                if b4:
                    lo, hi = b4
                    p3m = hi_of(b3)
                    if p3m > hi:
                        dve.tensor_copy(acc[:, :, hi:p3m],
                                        p4v[:, :, hi - l4:p3m - l4])
                    off, w = MOFFS[(bi, 3)]
                    dve.copy_predicated(acc[:, :, lo:hi],
                                        mask_t[:, :, off - moff:off - moff + w],
                                        p4v[:, :, lo - l4:hi - l4])
                if b3:
                    lo, hi = b3
                    p2m = hi_of(b2)
                    if p2m > hi:
                        dve.tensor_copy(acc[:, :, hi:p2m],
                                        s7b[:, :, hi - l3:p2m - l3])
                    off, w = MOFFS[(bi, 2)]
                    dve.copy_predicated(acc[:, :, lo:hi],
                                        mask_t[:, :, off - moff:off - moff + w],
                                        s7b[:, :, lo - l3:hi - l3])
                if b2:
                    lo, hi = b2
                    p1m = hi_of(b1)
                    if p1m > hi:
                        dve.tensor_copy(acc[:, :, hi:p1m],
                                        s4o[:, 3:31, hi + 4:p1m + 4])
                    off, w = MOFFS[(bi, 1)]
                    dve.copy_predicated(acc[:, :, lo:hi],
                                        mask_t[:, :, off - moff:off - moff + w],
                                        s4o[:, 3:31, lo + 4:hi + 4])
                if b1:
                    lo, hi = b1
                    if 112 > hi:
                        dve.tensor_copy(acc[:, :, hi:112],
                                        ee[:, 6:34, hi + 6:118])
                    off, w = MOFFS[(bi, 0)]
                    dve.copy_predicated(acc[:, :, lo:hi],
                                        mask_t[:, :, off - moff:off - moff + w],
                                        ee[:, 6:34, lo + 6:hi + 6])

                # ---- cast + store ----
                out_t = tO.tile([128, BH, 112], mybir.dt.float32, tag="out")
                act.copy(out_t[:, :, :], acc[:, :, :])
                nc.sync.dma_start(y0d[:, y0:y0 + BH, :], out_t[0:64, :, :])
                nc.sync.dma_start(y1d[:, y0:y0 + BH, :], out_t[64:128, :, :])

    return nc


def _get_nc():
    if "nc" not in _CACHED:
        nc = bass.Bass()
        _emit_kernel(nc)
        split_multi_waits(nc)
        _CACHED["nc"] = nc
    return _CACHED["nc"]


def _in_maps(x):
    maps = []
    for b in range(B):
        xb = np.ascontiguousarray(x[b], dtype=np.float32)
        xrb = np.ascontiguousarray(xb[:, :, ::-1])
        maps.append({"x": xb, "xr": xrb})
    return maps


def kernel(x: np.ndarray) -> np.ndarray:
    nc = _get_nc()
    res = run_bass_kernel_spmd(nc, _in_maps(x), core_ids=list(range(B)))
    out = np.empty((B, C, OUT, OUT), np.float32)
    for b, r in enumerate(res.results):
        out[b, :, :, 0:112] = r["y0d"]
        out[b, :, :, 112:224] = r["y1d"][:, :, ::-1]
    return out


# revision 8
# speedup vs baseline: 4.1043x; 1.1026x over previous
"""Trainium2 Bass kernel for CenterDependentPool2D.

Input  x: (8, 64, 448, 448) fp32  ->  Output: (8, 64, 224, 224) fp32.

Strategy (per core = one batch element, 64 channels):
  - Partition p = c + 64*wg. Position w in [0,112): wg0 -> out col w,
    wg1 -> out col 223-w (wg1 input is column-MIRRORED on the host via a
    second DRAM tensor xr, so both wgs see "outer edge at w=0, center at
    w=111" and every ring interval is a contiguous [lo,hi) slice).
  - All five ring windows (k in {2,8,14,20,26}, stride 2, reflect pad)
    decompose over pair-max arrays Ew[e]=max(x[2e],x[2e+1]) (stored at
    j=e+6) and Ow (odd pairs), pooled vertically to EE/OO, then a shared
    shifted-max pyramid per ring. Reflect padding == window clipping here,
    so out-of-range leaves are -BIG fills.
  - NEW vs v0: every pyramid op is restricted to the column interval where
    its ring can win (ring r lives in the annulus R_{r-1} < d < R_r, which
    in mirrored position space is one interval per band). The blend is
    per-band: the outer ring writes acc directly, inner rings do one plain
    segment copy (fp16 4x) plus one narrow copy_predicated boundary strip.
  - 8 bands x 28 rows. Rolling Ew/Ow raw-pair tiles (roll on scalar).
    fp16 pipeline after the first max; final cast on scalar engine.
"""

import math
import numpy as np

import concourse.bass as bass
import concourse.mybir as mybir
from concourse.tile import TileContext
from concourse.bass_utils import run_bass_kernel_spmd

# ---------------- problem constants ----------------
B, C, IN, OUT = 8, 64, 448, 224
BH = 28                   # out rows per band
NB = OUT // BH            # 8 bands
RADII = [60, 75, 90, 105]
NEG = -30000.0
EW = 124                  # E/O array width (j = position e + 6)
RE = 2 * BH + 24          # 80 raw rows resident per band
ITC = 16                  # itile chunk rows
DT = mybir.dt.float16

_CACHED = {}


# ---------------- geometry ----------------
def band_geometry():
    bands = []
    for i in range(NB):
        y0 = i * BH
        rings = []
        for R in RADII:
            bmin, bmax, present = 112, 0, False
            for yy in range(y0, y0 + BH):
                dy = abs(yy - 112)
                if dy < R:
                    present = True
                    s = math.sqrt(R * R - dy * dy)
                    bmin = min(bmin, 111 - s)
                    bmax = max(bmax, 112 - s)
                else:
                    bmax = 112
            if not present:
                rings.append(None)
            else:
                rings.append((max(0, math.floor(bmin)),
                              min(112, math.floor(bmax) + 1)))
        b1, b2, b3, b4 = rings
        hi = lambda r: r[1] if r else 112
        I5 = (0, hi(b4))
        I4 = (b4[0], hi(b3)) if b4 else None
        I3 = (b3[0], hi(b2)) if b3 else None
        I2 = (b2[0], hi(b1)) if b2 else None
        I1 = (b1[0], 112) if b1 else None
        bands.append(dict(y0=y0, strips=[b1, b2, b3, b4],
                          I=[I1, I2, I3, I4, I5]))
    return bands


def merge_ivs(ivs, gap=14):
    ivs = sorted([list(v) for v in ivs if v is not None])
    out = []
    for iv in ivs:
        if out and iv[0] <= out[-1][1] + gap:
            out[-1][1] = max(out[-1][1], iv[1])
        else:
            out.append(iv)
    return [tuple(v) for v in out]


BANDS = band_geometry()


def build_masks():
    """Packed per-band boundary-strip masks [128, 28, TOTW] u8; per band the
    ring blocks (r4,r3,r2,r1 order) are contiguous so one DMA per band."""
    yy = np.arange(OUT)
    blocks, offs, off = [], {}, 0
    for bi, bd in enumerate(BANDS):
        for ri in (3, 2, 1, 0):
            st = bd['strips'][ri]
            if st is None:
                continue
            lo, hi = st
            w = hi - lo
            R2 = RADII[ri] ** 2
            rows = yy[bd['y0']:bd['y0'] + BH]
            dy2 = (rows - 112) ** 2                       # [28]
            wpos = np.arange(lo, hi)
            m = np.zeros((128, BH, w), np.uint8)
            dx0 = (112 - wpos) ** 2
            dx1 = (111 - wpos) ** 2
            m[0:64] = (dy2[None, :, None] + dx0[None, None, :] < R2)
            m[64:128] = (dy2[None, :, None] + dx1[None, None, :] < R2)
            offs[(bi, ri)] = (off, w)
            blocks.append(m)
            off += w
    return np.concatenate(blocks, axis=2), offs


MASKS, MOFFS = build_masks()
MTOT = MASKS.shape[2]
MBAND = {}               # band -> (off, width) of its contiguous mask block
for bi in range(NB):
    pieces = [MOFFS[(bi, ri)] for ri in (3, 2, 1, 0) if (bi, ri) in MOFFS]
    MBAND[bi] = (pieces[0][0], sum(w for _, w in pieces))
MW = max(w for _, w in MBAND.values())


def split_multi_waits(nc):
    """walrus CoreV3Gen accepts at most 1 sync-wait per instruction; peel
    extras onto preceding NoOps."""
    n = 0
    for fn in nc.m.functions:
        for bb in fn.blocks:
            insts = list(bb.instructions)
            out = []
            for ins in insts:
                si = getattr(ins, "sync_info", None)
                if si is not None and len(si.on_wait) > 1:
                    waits = list(si.on_wait)
                    for k, w in enumerate(waits[:-1]):
                        nop = mybir.InstNoOp(
                            name=f"{ins.name}-waitsplit{k}",
                            engine=ins.engine, ins=[], outs=[])
                        nop.sync_info = mybir.SyncInfo(on_wait=[w], on_update=[])
                        out.append(nop)
                        n += 1
                    ins.sync_info = mybir.SyncInfo(
                        on_wait=[waits[-1]], on_update=list(si.on_update))
                out.append(ins)
            if n:
                bb.instructions = out
    return n


def _emit_kernel(nc: bass.Bass):
    x = nc.dram_tensor("x", [C, IN, IN], mybir.dt.float32, kind="ExternalInput")
    xr = nc.dram_tensor("xr", [C, IN, IN], mybir.dt.float32, kind="ExternalInput")
    y0d = nc.dram_tensor("y0d", [C, OUT, 112], mybir.dt.float32, kind="ExternalOutput")
    y1d = nc.dram_tensor("y1d", [C, OUT, 112], mybir.dt.float32, kind="ExternalOutput")
    rmask = nc.inline_tensor(MASKS, name="rmask")

    dve = nc.vector
    act = nc.scalar
    mx = mybir.AluOpType.max

    with TileContext(nc) as tc:
        with tc.tile_pool(name="pp", bufs=1) as pers, \
             tc.tile_pool(name="ts2", bufs=1) as tS2, \
             tc.tile_pool(name="ts4", bufs=1) as tS4, \
             tc.tile_pool(name="tmp", bufs=1) as tT, \
             tc.tile_pool(name="ts8", bufs=1) as tS8, \
             tc.tile_pool(name="fin", bufs=1) as tF, \
             tc.tile_pool(name="tac", bufs=1) as tA, \
             tc.tile_pool(name="tou", bufs=1) as tO, \
             tc.tile_pool(name="tmk", bufs=2) as tM, \
             tc.tile_pool(name="tit", bufs=3) as tIT:

            ewt = pers.tile([128, RE, EW], DT, tag="ewt")
            owt = pers.tile([128, RE, EW], DT, tag="owt")
            ee = pers.tile([128, 40, EW], DT, tag="ee")
            oo = pers.tile([128, 38, EW], DT, tag="oo")

            # initial NEG fill of the top reflect-pad rows
            nc.gpsimd.memset(ewt[:, 0:12, :], NEG)
            nc.gpsimd.memset(owt[:, 0:12, :], NEG)

            # per-band oo/owt column intervals (chains only need these cols)
            def oo_cols(bj):
                Ii = BANDS[bj]['I']
                ivs = []
                if Ii[3]:
                    ivs.append((Ii[3][0] + 1, min(EW, Ii[3][1] + 10)))
                if Ii[1]:
                    ivs.append((Ii[1][0] + 4, min(EW, Ii[1][1] + 7)))
                return merge_ivs(ivs)

            OC = [oo_cols(bj) for bj in range(NB)]
            OCU = [merge_ivs(OC[bj] + (OC[bj + 1] if bj + 1 < NB else []))
                   for bj in range(NB)]

            chunk_state = {"idx": 0}
            pend = {}          # band -> list of (itile, k, n)
            mtiles = {}        # band -> mask tile

            def chunk_dma(ga, n):
                """Allocate a fresh pool itile (rotation waits for the previous
                tenant's readers -> no DMA-clobber race), set its NEG pads on
                gpsimd, then DMA the chunk rows."""
                itile = tIT.tile([128, ITC, 250], mybir.dt.float32, tag="it")
                nc.gpsimd.memset(itile[:, 0:n, 0:13], NEG)
                nc.gpsimd.memset(itile[64:128, 0:n, 249:250], NEG)
                nc.sync.dma_start(itile[0:64, 0:n, 13:250],
                                  x[:, ga:ga + n, 0:237])
                nc.scalar.dma_start(itile[64:128, 0:n, 13:249],
                                    xr[:, ga:ga + n, 0:236])
                return itile

            def emit_dma(bj):
                """Issue the input DMAs + mask DMA for band bj."""
                gg0 = 2 * BANDS[bj]['y0'] - 12
                klo = 12 if bj == 0 else 24
                khi = min(RE, IN - gg0)
                lst = []
                k = klo
                while k < khi:
                    n = min(ITC, khi - k)
                    lst.append((chunk_dma(gg0 + k, n), k, n))
                    k += n
                pend[bj] = lst
                moff, mw = MBAND[bj]
                mask_t = tM.tile([128, BH, MW], mybir.dt.uint8, tag="mk")
                nc.scalar.dma_start(mask_t[:, :, 0:mw], rmask[:, :, moff:moff + mw])
                mtiles[bj] = mask_t

            emit_dma(0)
            for bi, bd in enumerate(BANDS):
                y0 = bd['y0']
                I1, I2, I3, I4, I5 = bd['I']
                b1, b2, b3, b4 = bd['strips']
                g0 = 2 * y0 - 12
                khi = min(RE, IN - g0)

                # ---- fresh pair rows (input DMA'd during previous band) ----
                for itile, k, n in pend.pop(bi):
                    dve.tensor_tensor(ewt[:, k:k + n, :],
                                      itile[:, 0:n, 1:249:2],
                                      itile[:, 0:n, 2:250:2], mx)
                    for lo, hi in OCU[bi]:
                        dve.tensor_tensor(owt[:, k:k + n, lo:hi],
                                          itile[:, 0:n, 2 + 2 * lo:2 + 2 * hi:2],
                                          itile[:, 0:n, 3 + 2 * lo:3 + 2 * hi:2],
                                          mx)
                if khi < RE:
                    nc.gpsimd.memset(ewt[:, khi:RE, :], NEG)
                    nc.gpsimd.memset(owt[:, khi:RE, :], NEG)
                mask_t = mtiles.pop(bi)
                moff, mw = MBAND[bi]

                # ---- ee / oo ----
                dve.tensor_tensor(ee[:, 0:40, :], ewt[:, 0:80:2, :],
                                  ewt[:, 1:80:2, :], mx)
                need_oo = I4 or I2
                for lo, hi in OC[bi]:
                    dve.tensor_tensor(oo[:, 0:38, lo:hi], owt[:, 3:79:2, lo:hi],
                                      owt[:, 4:80:2, lo:hi], mx)

                # ---- prefetch band bi+1: roll (scalar) + input/mask DMA ----
                if bi + 1 < NB:
                    act.copy(ewt[:, 0:24, :], ewt[:, 56:80, :])
                    for lo, hi in OC[bi + 1]:
                        act.copy(owt[:, 0:24, lo:hi], owt[:, 56:80, lo:hi])
                    emit_dma(bi + 1)

                # ---- acc ----
                acc = tA.tile([128, BH, 112], DT, tag="acc")

                # ================= EE chain =================
                l5, h5 = I5
                iv2 = [(0, min(EW, h5 + 12))]
                iv4 = [(0, min(EW, h5 + 11))]
                if I3:
                    l3, h3 = I3
                    iv2.append((l3 + 3, min(EW, h3 + 9)))
                    iv4.append((l3 + 3, min(EW, h3 + 8)))
                iv2 = merge_ivs(iv2)
                iv4 = merge_ivs(iv4)

                s2 = tS2.tile([128, 39, EW], DT, tag="s2")
                for lo, hi in iv2:
                    a2 = tT.tile([128, 39, EW], DT, tag="tmp")
                    dve.tensor_tensor(a2[:, :, lo:hi], ee[:, 0:39, lo:hi],
                                      ee[:, 1:40, lo:hi], mx)
                    dve.tensor_tensor(s2[:, :, lo:hi - 1], a2[:, :, lo:hi - 1],
                                      a2[:, :, lo + 1:hi], mx)
                s4 = tS4.tile([128, 37, EW], DT, tag="s4")
                for lo, hi in iv4:
                    a4 = tT.tile([128, 37, EW], DT, tag="tmp")
                    dve.tensor_tensor(a4[:, :, lo:hi], s2[:, 0:37, lo:hi],
                                      s2[:, 2:39, lo:hi], mx)
                    dve.tensor_tensor(s4[:, :, lo:hi - 2], a4[:, :, lo:hi - 2],
                                      a4[:, :, lo + 2:hi], mx)
                # P5 finals
                w8 = min(EW, h5 + 9)
                a8 = tT.tile([128, 33, EW], DT, tag="tmp")
                dve.tensor_tensor(a8[:, :, 0:w8], s4[:, 0:33, 0:w8],
                                  s4[:, 4:37, 0:w8], mx)
                s8 = tS8.tile([128, 33, EW], DT, tag="s8")
                dve.tensor_tensor(s8[:, :, 0:w8 - 4], a8[:, :, 0:w8 - 4],
                                  a8[:, :, 4:w8], mx)
                v13 = tF.tile([128, BH, EW], DT, tag="fin")
                dve.tensor_tensor(v13[:, :, 0:w8 - 4], s8[:, 0:28, 0:w8 - 4],
                                  s8[:, 5:33, 0:w8 - 4], mx)
                # s13 writes acc[0:h5) directly
                dve.tensor_tensor(acc[:, :, 0:h5], v13[:, :, 0:h5],
                                  v13[:, :, 5:h5 + 5], mx)
                if I3:
                    u = tT.tile([128, BH, EW], DT, tag="tmp")
                    dve.tensor_tensor(u[:, :, l3 + 3:h3 + 6],
                                      s4[:, 3:31, l3 + 3:h3 + 6],
                                      s4[:, 6:34, l3 + 3:h3 + 6], mx)
                    s7b = tF.tile([128, BH, 80], DT, tag="s7b")
                    dve.tensor_tensor(s7b[:, :, 0:h3 - l3],
                                      u[:, :, l3 + 3:h3 + 3],
                                      u[:, :, l3 + 6:h3 + 6], mx)

                # ================= OO chain =================
                if need_oo:
                    ivo2, ivo4 = [], []
                    if I4:
                        l4, h4 = I4
                        ivo2.append((l4 + 1, min(EW, h4 + 10)))
                        ivo4.append((l4 + 1, min(EW, h4 + 9)))
                    if I2:
                        l2, h2 = I2
                        ivo2.append((l2 + 4, min(EW, h2 + 7)))
                        ivo4.append((l2 + 4, min(EW, h2 + 6)))
                    ivo2 = merge_ivs(ivo2)
                    ivo4 = merge_ivs(ivo4)
                    s2o = tS2.tile([128, 37, EW], DT, tag="s2o")
                    for lo, hi in ivo2:
                        a2o = tT.tile([128, 37, EW], DT, tag="tmp")
                        dve.tensor_tensor(a2o[:, :, lo:hi], oo[:, 0:37, lo:hi],
                                          oo[:, 1:38, lo:hi], mx)
                        dve.tensor_tensor(s2o[:, :, lo:hi - 1],
                                          a2o[:, :, lo:hi - 1],
                                          a2o[:, :, lo + 1:hi], mx)
                    s4o = tS4.tile([128, 35, EW], DT, tag="s4o")
                    for lo, hi in ivo4:
                        a4o = tT.tile([128, 35, EW], DT, tag="tmp")
                        dve.tensor_tensor(a4o[:, :, lo:hi], s2o[:, 0:35, lo:hi],
                                          s2o[:, 2:37, lo:hi], mx)
                        dve.tensor_tensor(s4o[:, :, lo:hi - 2],
                                          a4o[:, :, lo:hi - 2],
                                          a4o[:, :, lo + 2:hi], mx)
                    if I4:
                        w8o = min(EW, h4 + 7)
                        a8o = tT.tile([128, 31, EW], DT, tag="tmp")
                        dve.tensor_tensor(a8o[:, :, l4 + 1:w8o],
                                          s4o[:, 0:31, l4 + 1:w8o],
                                          s4o[:, 4:35, l4 + 1:w8o], mx)
                        s8o = tS8.tile([128, 31, EW], DT, tag="s8")
                        dve.tensor_tensor(s8o[:, :, 0:w8o - l4 - 5],
                                          a8o[:, :, l4 + 1:w8o - 4],
                                          a8o[:, :, l4 + 5:w8o], mx)
                        # s8o col q == j = l4+1+q, valid q in [0, h4+2-l4)
                        v10 = tF.tile([128, BH, EW], DT, tag="fin")
                        nq = h4 + 2 - l4
                        dve.tensor_tensor(v10[:, :, 0:nq], s8o[:, 0:28, 0:nq],
                                          s8o[:, 2:30, 0:nq], mx)
                        p4v = tF.tile([128, BH, 80], DT, tag="p4v")
                        dve.tensor_tensor(p4v[:, :, 0:h4 - l4],
                                          v10[:, :, 0:h4 - l4],
                                          v10[:, :, 2:h4 - l4 + 2], mx)

                # ================= blend =================
                hi_of = lambda r: r[1] if r else 112
                if b4:
                    lo, hi = b4
                    p3m = hi_of(b3)
                    if p3m > hi:
                        dve.tensor_copy(acc[:, :, hi:p3m],
                                        p4v[:, :, hi - l4:p3m - l4])
                    off, w = MOFFS[(bi, 3)]
                    dve.copy_predicated(acc[:, :, lo:hi],
                                        mask_t[:, :, off - moff:off - moff + w],
                                        p4v[:, :, lo - l4:hi - l4])
                if b3:
                    lo, hi = b3
                    p2m = hi_of(b2)
                    if p2m > hi:
                        dve.tensor_copy(acc[:, :, hi:p2m],
                                        s7b[:, :, hi - l3:p2m - l3])
                    off, w = MOFFS[(bi, 2)]
                    dve.copy_predicated(acc[:, :, lo:hi],
                                        mask_t[:, :, off - moff:off - moff + w],
                                        s7b[:, :, lo - l3:hi - l3])
                if b2:
                    lo, hi = b2
                    p1m = hi_of(b1)
                    if p1m > hi:
                        dve.tensor_copy(acc[:, :, hi:p1m],
                                        s4o[:, 3:31, hi + 4:p1m + 4])
                    off, w = MOFFS[(bi, 1)]
                    dve.copy_predicated(acc[:, :, lo:hi],
                                        mask_t[:, :, off - moff:off - moff + w],
                                        s4o[:, 3:31, lo + 4:hi + 4])
                if b1:
                    lo, hi = b1
                    if 112 > hi:
                        dve.tensor_copy(acc[:, :, hi:112],
                                        ee[:, 6:34, hi + 6:118])
                    off, w = MOFFS[(bi, 0)]
                    dve.copy_predicated(acc[:, :, lo:hi],
                                        mask_t[:, :, off - moff:off - moff + w],
                                        ee[:, 6:34, lo + 6:hi + 6])

                # ---- cast + store ----
                out_t = tO.tile([128, BH, 112], mybir.dt.float32, tag="out")
                act.copy(out_t[:, :, :], acc[:, :, :])
                nc.sync.dma_start(y0d[:, y0:y0 + BH, :], out_t[0:64, :, :])
                nc.sync.dma_start(y1d[:, y0:y0 + BH, :], out_t[64:128, :, :])

    return nc


def _get_nc():
    if "nc" not in _CACHED:
        nc = bass.Bass()
        _emit_kernel(nc)
        split_multi_waits(nc)
        _CACHED["nc"] = nc
    return _CACHED["nc"]


def _in_maps(x):
    maps = []
    for b in range(B):
        xb = np.ascontiguousarray(x[b], dtype=np.float32)
        xrb = np.ascontiguousarray(xb[:, :, ::-1])
        maps.append({"x": xb, "xr": xrb})
    return maps


def kernel(x: np.ndarray) -> np.ndarray:
    nc = _get_nc()
    res = run_bass_kernel_spmd(nc, _in_maps(x), core_ids=list(range(B)))
    out = np.empty((B, C, OUT, OUT), np.float32)
    for b, r in enumerate(res.results):
        out[b, :, :, 0:112] = r["y0d"]
        out[b, :, :, 112:224] = r["y1d"][:, :, ::-1]
    return out
